# revision 1
# baseline (speedup 1.0000x reference)
"""Trainium2 Bass kernel for nn_AttentionBlock (B=2, T=2048, D=1024, H=16,
Dh=64, Ff=4096), SPMD across 8 NeuronCores in one NEFF launch.

Sharding:
  - Phase 1+2 (QKV projection + attention): 2 heads per core. The alibi
    tensor (256 MiB total) is read bf16, 2 heads per core.
  - AllToAll (1 MiB/core, bf16) re-shards attention output from heads to
    tokens.
  - Phase 3 (out-proj + residual + LayerNorm + MLP): 512 tokens per core.

Numerics:
  - Matmuls on the q/k path use float32r (TF32-like, ~1.5e-4) fed straight
    from fp32 HBM data; bf16 elsewhere (weights, alibi).
  - Attention computes transposed scores S^T(k,q) in 1024-wide tiles:
    Q.K^T runs in float32r, ScalarE computes exp(PSUM)->bf16, and the
    host-precomputed exp(alibi) (bf16) multiplies in on VectorE
    (exp(s+a) = exp(s)*exp(a)); the softmax denominator falls out of a
    ones column appended to V in the attn@v matmul; 1/denom is broadcast
    across partitions with gpsimd.partition_broadcast. Alibi tiles are
    cached per (head, q-chunk) round so both batches share one load.
  - Host-side algebraic folds: 1/sqrt(Dh) into w_q, ln2_w into w_mlp_in,
    b_mlp_in via gelu's per-partition bias, b_mlp_out into a second copy
    of the residual.

kernel(**inputs) takes FULL unsharded inputs, returns the FULL output.
"""

import sys

for _p in ("/opt/trn_rl_repo", "/root/.axon_site/_ro/trn_rl_repo"):
    if _p not in sys.path:
        sys.path.insert(0, _p)

import numpy as np
import ml_dtypes

import concourse.bass as bass
import concourse.tile as tile
from concourse import bacc, mybir
from concourse.bass_utils import run_bass_kernel_spmd
from concourse.masks import make_identity

BF16 = ml_dtypes.bfloat16

B, T, D, H, Dh, FF = 2, 2048, 1024, 16, 64, 4096
NTOK = B * T            # 4096
NCORES = 8
CHUNK = NTOK // NCORES  # 512 tokens per core
HPC = H // NCORES       # 2 heads per core

F32 = mybir.dt.float32
F32R = mybir.dt.float32r
BF = mybir.dt.bfloat16
AF = mybir.ActivationFunctionType

_COMPILED = None


def _build(sim1=False):
    nc = bacc.Bacc("TRN2", target_bir_lowering=False, debug=False,
                   num_devices=1 if sim1 else NCORES)

    # ---- kernel I/O (per core) ----
    xT_io = nc.dram_tensor("xT", [D, NTOK], F32R, kind="ExternalInput").ap()
    wqkvT_io = nc.dram_tensor("wqkvT", [D, 384], F32R, kind="ExternalInput").ap()
    alibiT_io = nc.dram_tensor("alibiT", [HPC, T, T], BF, kind="ExternalInput").ap()
    w_outT_io = nc.dram_tensor("w_outT", [D, D], BF, kind="ExternalInput").ap()
    x_res_io = nc.dram_tensor("x_res", [CHUNK, D], F32, kind="ExternalInput").ap()
    x_res_b_io = nc.dram_tensor("x_res_b", [CHUNK, D], F32, kind="ExternalInput").ap()
    # packed as [p, ff, kk, fin] = w_mlp_in_eff[ff*128+fin, kk*128+p]
    w_inP_io = nc.dram_tensor("w_inP", [128, 32, 8, 128], BF, kind="ExternalInput").ap()
    b_inT_io = nc.dram_tensor("b_inT", [128, 32], F32, kind="ExternalInput").ap()
    w_mlp_outT_io = nc.dram_tensor("w_mlp_outT", [FF, D], BF, kind="ExternalInput").ap()
    out_io = nc.dram_tensor("out", [CHUNK, D], F32, kind="ExternalOutput").ap()

    # ---- internal DRAM ----
    cc_send = nc.dram_tensor("cc_send", [D, CHUNK], BF)
    cc_recv = nc.dram_tensor("cc_recv", [D, CHUNK], BF)

    KT = T // 128   # 16 k-tiles per batch

    with tile.TileContext(nc) as tc:
        with tc.tile_pool(name="consts", bufs=1) as consts:
            identb = consts.tile([128, 128], BF, tag="identb")
            make_identity(nc, identb[:])
            identf = consts.tile([128, 128], F32, tag="identf")
            make_identity(nc, identf[:])
            identr = consts.tile([128, 128], F32R, tag="identr")
            nc.vector.tensor_copy(identr[:], identf[:])
            warm_f = consts.tile([128, 512], F32, tag="warm_f")
            nc.vector.memset(warm_f[:], 0.5)
            warm_rhs = consts.tile([128, 512], F32R, tag="warm_rhs")
            nc.vector.tensor_copy(warm_rhs[:], warm_f[:])


            with tc.tile_pool(name="qkv", bufs=1) as qkv:
                # per-batch q/k/v so batch-1 projection overlaps batch-0
                # attention without false dependencies
                qTs, kTs, vs = [], [], []
                for b in range(2):
                    qTb = qkv.tile([128, T], F32R, tag=f"qT{b}", name=f"qT{b}")
                    kTb = qkv.tile([128, T], F32R, tag=f"kT{b}", name=f"kT{b}")
                    vb = qkv.tile([128, 16, 2, 65], BF, tag=f"v{b}",
                                  name=f"v{b}")
                    nc.vector.memset(vb[:, :, :, 64:65], 1.0)
                    qTs.append(qTb); kTs.append(kTb); vs.append(vb)
                # yn[hl][b*2+qc] covers tokens [b*T + qc*1024, ...)
                yn = [[qkv.tile([64, 1024], BF, tag=f"yn{hl}_{i}",
                                name=f"yn{hl}_{i}") for i in range(4)]
                      for hl in range(2)]

                with tc.tile_pool(name="p1x", bufs=1) as p1x, \
                     tc.tile_pool(name="p1w", bufs=1) as p1w, \
                     tc.tile_pool(name="p1ps", bufs=4, space="PSUM") as p1ps, \
                     tc.tile_pool(name="p1t", bufs=3) as p1t, \
                     tc.tile_pool(name="p1pt", bufs=2, space="PSUM") as p1pt:
                    wq = []
                    for kk in range(8):
                        w = p1w.tile([128, 384], F32R, tag=f"wq{kk}")
                        nc.sync.dma_start(w[:], wqkvT_io[kk * 128:(kk + 1) * 128, :])
                        wq.append(w)
                    def proj_pass(b):
                        qT, kT, v_all = qTs[b], kTs[b], vs[b]
                        with nc.named_scope(f"qkvproj{b}"):
                            xts = [p1x.tile([128, 2048], F32R,
                                            tag=f"xt{kk}", name=f"xt{kk}_{b}")
                                   for kk in range(8)]
                            for cc4 in range(4):
                                for kk in range(8):
                                    nc.sync.dma_start(
                                        xts[kk][:, cc4 * 512:(cc4 + 1) * 512],
                                        xT_io[kk * 128:(kk + 1) * 128,
                                              b * 2048 + cc4 * 512:
                                              b * 2048 + (cc4 + 1) * 512])
                            for t in range(4):
                                for m in range(3):   # q, k, v
                                    ps = p1ps.tile([128, 512], F32, tag="proj",
                                                   name=f"proj{b}_{t}_{m}")
                                    for kk in range(8):
                                        nc.tensor.matmul(
                                            ps[:],
                                            wq[kk][:, m * 128:(m + 1) * 128],
                                            xts[kk][:, t * 512:(t + 1) * 512],
                                            start=(kk == 0), stop=(kk == 7))
                                    if m == 0:
                                        nc.vector.tensor_copy(
                                            qT[:, t * 512:(t + 1) * 512], ps[:])
                                    elif m == 1:
                                        nc.vector.tensor_copy(
                                            kT[:, t * 512:(t + 1) * 512], ps[:])
                                    else:
                                        vt = p1t.tile([128, 512], F32R, tag="vt",
                                                      name=f"vt{b}_{t}")
                                        nc.vector.tensor_copy(vt[:], ps[:])
                                        for j in range(4):
                                            ti = t * 4 + j
                                            pt = p1pt.tile([128, 128], F32R,
                                                           tag="pt",
                                                           name=f"pt{b}_{ti}")
                                            nc.tensor.transpose(
                                                pt[:],
                                                vt[:, j * 128:(j + 1) * 128],
                                                identr[:])
                                            nc.vector.tensor_copy(
                                                v_all[:, ti, :, 0:64],
                                                pt[:].rearrange(
                                                    "p (a b) -> p a b", a=2))

                    for wi_ in range(14):
                        wps = p1pt.tile([128, 512], F32, tag="pt",
                                        name=f"warms{wi_}")
                        nc.tensor.matmul(wps[:], identr[:], warm_rhs[:],
                                         start=True, stop=True)
                    proj_pass(0)
                    proj_pass(1)

                with nc.named_scope("attn"), \
                     tc.tile_pool(name="alb", bufs=24) as albp, \
                     tc.tile_pool(name="exps", bufs=8) as expp, \
                     tc.tile_pool(name="sps", bufs=2, space="PSUM") as spsp, \
                     tc.tile_pool(name="yups", bufs=2, space="PSUM") as yupp, \
                     tc.tile_pool(name="nrm", bufs=4) as nrmp:
                    al_cache = {}

                    def attn_pass(hl, qc, b, early_release=False):
                        if (hl, qc) not in al_cache:
                            al_cache[(hl, qc)] = [
                                albp.tile([128, 1024], BF, tag="al",
                                          name=f"al{hl}_{qc}_{kt}")
                                for kt in range(KT)]
                        als = al_cache[(hl, qc)]
                        yu = yupp.tile([65, 1024], F32, tag="yu",
                                       name=f"yu{hl}_{qc}_{b}")
                        for kt in range(KT):
                            if b == 0:
                                nc.sync.dma_start(
                                    als[kt][:],
                                    alibiT_io[hl, kt * 128:(kt + 1) * 128,
                                              qc * 1024:(qc + 1) * 1024])
                            sp = spsp.tile([128, 1024], F32, tag="sp",
                                           name=f"sp{hl}_{qc}_{b}_{kt}")
                            for h2 in range(2):
                                nc.tensor.matmul(
                                    sp[:, h2 * 512:(h2 + 1) * 512],
                                    kTs[b][hl * 64:(hl + 1) * 64,
                                           kt * 128:(kt + 1) * 128],
                                    qTs[b][hl * 64:(hl + 1) * 64,
                                           qc * 1024 + h2 * 512:
                                           qc * 1024 + (h2 + 1) * 512],
                                    start=True, stop=True)
                            ex0 = expp.tile([128, 1024], BF, tag="ex0",
                                            name=f"ex0_{hl}_{qc}_{b}_{kt}")
                            nc.scalar.activation(ex0[:], sp[:], AF.Exp)
                            ex = expp.tile([128, 1024], BF, tag="ex",
                                           name=f"ex_{hl}_{qc}_{b}_{kt}")
                            nc.vector.tensor_mul(ex[:], ex0[:], als[kt][:])
                            for h2 in range(2):
                                nc.tensor.matmul(
                                    yu[:, h2 * 512:(h2 + 1) * 512],
                                    vs[b][:, kt, hl, :],
                                    ex[:, h2 * 512:(h2 + 1) * 512],
                                    start=(kt == 0), stop=(kt == KT - 1))
                        if early_release:
                            # free the PSUM bank fast so the boundary
                            # warmth-bridge matmuls can start
                            yuc = nrmp.tile([65, 1024], F32, tag="yuc",
                                            name=f"yuc{hl}_{qc}_{b}")
                            nc.vector.tensor_copy(yuc[:], yu[:])
                            yu_src = yuc
                        else:
                            yu_src = yu
                        rec = nrmp.tile([1, 1024], F32, tag="rec",
                                        name=f"rec{hl}_{qc}_{b}")
                        nc.vector.reciprocal(rec[:], yu_src[64:65, :])
                        bc = nrmp.tile([64, 1024], F32, tag="bc",
                                       name=f"bc{hl}_{qc}_{b}")
                        nc.gpsimd.partition_broadcast(bc[:], rec[:])
                        nc.vector.tensor_mul(
                            yn[hl][b * 2 + qc][:], yu_src[0:64, :], bc[:])
                        i = b * 2 + qc
                        nc.sync.dma_start(
                            bass.AP(tensor=cc_send,
                                    offset=(2 * i * 128 + hl * 64) * 512,
                                    ap=[[512, 64], [128 * 512, 2], [1, 512]]),
                            yn[hl][i][:].rearrange("p (h c) -> p h c", h=2))

                    for hl in range(2):
                        for qc in range(2):
                            for b in range(2):
                                attn_pass(hl, qc, b)

                with nc.named_scope("a2a"):
                    if sim1:
                        nc.sync.dma_start(cc_recv[:], cc_send[:])
                    else:
                        nc.gpsimd.collective_compute(
                            "AllToAll", mybir.AluOpType.bypass,
                            replica_groups=[list(range(NCORES))],
                            ins=[cc_send[:]], outs=[cc_recv[:]])

            # ---------------- phase 3: out-proj + LN + MLP ----------------
            with nc.named_scope("mlp"), \
                 tc.tile_pool(name="p3w", bufs=1) as p3w, \
                 tc.tile_pool(name="p3acc", bufs=2, space="PSUM") as p3acc, \
                 tc.tile_pool(name="p3mo", bufs=4, space="PSUM") as p3mo, \
                 tc.tile_pool(name="p3pt", bufs=2, space="PSUM") as p3pt, \
                 tc.tile_pool(name="p3sb", bufs=1) as p3sb, \
                 tc.tile_pool(name="p3r", bufs=3) as p3r, \
                 tc.tile_pool(name="p3s", bufs=4) as p3s, \
                 tc.tile_pool(name="mlpw", bufs=8) as mlpw:
                for wi_ in range(60):
                    wps = p3pt.tile([128, 512], F32, tag="pt3",
                                    name=f"warm{wi_}")
                    nc.tensor.matmul(wps[:], identr[:], warm_rhs[:],
                                     start=True, stop=True)
                yrT = p3w.tile([128, 8, 512], BF, tag="yrT")
                nc.scalar.dma_start(
                    yrT[:], bass.AP(tensor=cc_recv, offset=0,
                                    ap=[[512, 128], [128 * 512, 8], [1, 512]]))
                yrecv = [yrT[:, kk, :] for kk in range(8)]
                wout = []
                for kk in range(8):
                    wo = p3w.tile([128, D], BF, tag=f"wo{kk}")
                    nc.sync.dma_start(wo[:], w_outT_io[kk * 128:(kk + 1) * 128, :])
                    wout.append(wo)
                b_in = p3sb.tile([128, 32], F32, tag="b_in")
                nc.sync.dma_start(b_in[:], b_inT_io[:])

                y_sb = p3sb.tile([128, 4, D], F32, tag="y_sb")
                y2_sb = p3sb.tile([128, 4, D], F32, tag="y2_sb")
                x_res_r = x_res_io.rearrange("(t p) d -> p t d", p=128)
                x_res_b_r = x_res_b_io.rearrange("(t p) d -> p t d", p=128)
                for tt in range(4):
                    xr = p3r.tile([128, D], F32, tag="xr")
                    nc.sync.dma_start(xr[:], x_res_r[:, tt, :])
                    xrb = p3r.tile([128, D], F32, tag="xrb")
                    nc.sync.dma_start(xrb[:], x_res_b_r[:, tt, :])
                    for dc in range(2):
                        ps = p3acc.tile([128, 512], F32, tag="acc")
                        for kk in range(8):
                            nc.tensor.matmul(
                                ps[:], yrecv[kk][:, tt * 128:(tt + 1) * 128],
                                wout[kk][:, dc * 512:(dc + 1) * 512],
                                start=(kk == 0), stop=(kk == 7))
                        nc.vector.tensor_add(
                            y_sb[:, tt, dc * 512:(dc + 1) * 512], ps[:],
                            xr[:, dc * 512:(dc + 1) * 512])
                        nc.vector.tensor_add(
                            y2_sb[:, tt, dc * 512:(dc + 1) * 512], ps[:],
                            xrb[:, dc * 512:(dc + 1) * 512])

                # LayerNorm -> h_norm (bf16) -> transpose -> hT (D-major)
                hT = p3sb.tile([128, 8, 512], BF, tag="hT")
                for tt in range(4):
                    stats = p3s.tile([128, 2, 6], F32, tag="stats")
                    for g in range(2):
                        nc.vector.bn_stats(
                            stats[:, g, :],
                            y_sb[:, tt, g * 512:(g + 1) * 512])
                    mv = p3s.tile([128, 2], F32, tag="mv")
                    nc.vector.bn_aggr(mv[:], stats[:])
                    eps = p3s.tile([128, 1], F32, tag="eps")
                    nc.vector.memset(eps[:], 1e-5)
                    sd = p3s.tile([128, 1], F32, tag="sd")
                    nc.scalar.activation(sd[:], mv[:, 1:2], AF.Sqrt,
                                         bias=eps[:], scale=1.0)
                    rstd = p3s.tile([128, 1], F32, tag="rstd")
                    nc.vector.reciprocal(rstd[:], sd[:])
                    nb = p3s.tile([128, 1], F32, tag="nb")
                    nc.vector.tensor_mul(nb[:], mv[:, 0:1], rstd[:])
                    nb2 = p3s.tile([128, 1], F32, tag="nb2")
                    nc.scalar.mul(nb2[:], nb[:], -1.0)
                    hn = p3r.tile([128, D], BF, tag="hn")
                    nc.scalar.activation(hn[:], y_sb[:, tt, :], AF.Identity,
                                         bias=nb2[:], scale=rstd[:])
                    for dc in range(8):
                        pt = p3pt.tile([128, 128], BF, tag="pt3")
                        nc.tensor.transpose(
                            pt[:], hn[:, dc * 128:(dc + 1) * 128], identb[:])
                        nc.vector.tensor_copy(
                            hT[:, dc, tt * 128:(tt + 1) * 128], pt[:])

                # MLP in + gelu -> hmT (Ff-major bf16)
                hmT = p3sb.tile([128, 32, 512], BF, tag="hmT")
                for ff in range(32):
                    wi = mlpw.tile([128, 8, 128], BF, tag="wi")
                    nc.sync.dma_start(wi[:], w_inP_io[:, ff, :, :])
                    ps = p3acc.tile([128, 512], F32, tag="acc")
                    for kk in range(8):
                        nc.tensor.matmul(ps[:], wi[:, kk, :], hT[:, kk, :],
                                         start=(kk == 0), stop=(kk == 7))
                    nc.scalar.activation(hmT[:, ff, :], ps[:], AF.Gelu,
                                         bias=b_in[:, ff:ff + 1], scale=1.0)

                # MLP out + final residual
                out_r = out_io.rearrange("(t p) d -> p t d", p=128)
                for dc in range(2):
                    pss = [p3mo.tile([128, 512], F32, tag="mo",
                                     name=f"mo{dc}_{i}") for i in range(4)]
                    for ff in range(32):
                        wo2 = mlpw.tile([128, 512], BF, tag="wo2")
                        nc.sync.dma_start(
                            wo2[:], w_mlp_outT_io[ff * 128:(ff + 1) * 128,
                                                  dc * 512:(dc + 1) * 512])
                        for tt in range(4):
                            nc.tensor.matmul(
                                pss[tt][:],
                                hmT[:, ff, tt * 128:(tt + 1) * 128], wo2[:],
                                start=(ff == 0), stop=(ff == 31))
                    for tt in range(4):
                        fin = p3s.tile([128, 512], F32, tag="fin")
                        nc.vector.tensor_add(
                            fin[:], pss[tt][:],
                            y2_sb[:, tt, dc * 512:(dc + 1) * 512])
                        nc.sync.dma_start(
                            out_r[:, tt, dc * 512:(dc + 1) * 512], fin[:])

    nc.compile()
    return nc


def _host_prep(x, alibi, ln1_w, w_qkv, w_out, ln2_w, w_mlp_in, b_mlp_in,
               w_mlp_out, b_mlp_out):
    f32 = np.float32
    x = np.asarray(x, f32)
    x_flat = np.ascontiguousarray(x.reshape(NTOK, D))
    xT = np.ascontiguousarray(x_flat.T)
    w_qkv = np.asarray(w_qkv, f32)
    w_out = np.asarray(w_out, f32)
    w_mlp_in = np.asarray(w_mlp_in, f32)
    w_mlp_out = np.asarray(w_mlp_out, f32)
    b_mlp_in = np.asarray(b_mlp_in, f32)
    b_mlp_out = np.asarray(b_mlp_out, f32)
    ln2_w = np.asarray(ln2_w, f32)
    alibi = np.asarray(alibi, f32)

    w_outT = np.ascontiguousarray(w_out.T).astype(BF16)
    w_in_eff = w_mlp_in * ln2_w[None, :]          # (FF, D)
    # packed [p, ff, kk, fin] = w_in_eff[ff*128+fin, kk*128+p]
    w_inP = np.ascontiguousarray(
        w_in_eff.reshape(32, 128, 8, 128).transpose(3, 0, 2, 1)).astype(BF16)
    w_mlp_outT = np.ascontiguousarray(w_mlp_out.T).astype(BF16)
    b_inT = np.ascontiguousarray(b_mlp_in.reshape(32, 128).T)

    in_maps = []
    for c in range(NCORES):
        h0 = HPC * c
        qrows = w_qkv[h0 * Dh:(h0 + HPC) * Dh] / np.sqrt(np.float32(Dh))
        krows = w_qkv[H * Dh + h0 * Dh:H * Dh + (h0 + HPC) * Dh]
        vrows = w_qkv[2 * H * Dh + h0 * Dh:2 * H * Dh + (h0 + HPC) * Dh]
        wqkvT = np.ascontiguousarray(np.concatenate([qrows, krows, vrows], 0).T)
        alibiT = np.exp(np.ascontiguousarray(
            np.transpose(alibi[0, h0:h0 + HPC], (0, 2, 1)))).astype(BF16)
        x_res = np.ascontiguousarray(x_flat[c * CHUNK:(c + 1) * CHUNK])
        x_res_b = x_res + b_mlp_out[None, :]
        in_maps.append({
            "xT": xT, "wqkvT": wqkvT, "alibiT": alibiT, "w_outT": w_outT,
            "x_res": x_res, "x_res_b": x_res_b, "w_inP": w_inP,
            "b_inT": b_inT, "w_mlp_outT": w_mlp_outT,
        })
    return in_maps


def _get_compiled():
    global _COMPILED
    if _COMPILED is None:
        _COMPILED = _build()
    return _COMPILED


def kernel(_trace=False, **inputs):
    nc = _get_compiled()
    in_maps = _host_prep(**inputs)
    res = None
    for attempt in range(3):
        try:
            res = run_bass_kernel_spmd(nc, in_maps,
                                       core_ids=list(range(NCORES)),
                                       trace=_trace)
            break
        except Exception:
            if attempt == 2:
                raise
    out = np.concatenate([res.results[c]["out"] for c in range(NCORES)], 0)
    out = out.reshape(B, T, D).astype(np.float32)
    if _trace:
        return out, res
    return out



# revision 14
# speedup vs baseline: 1.0997x; 1.0997x over previous
"""Trainium2 Bass kernel for nn_AttentionBlock (B=2, T=2048, D=1024, H=16,
Dh=64, Ff=4096), SPMD across 8 NeuronCores in one NEFF launch.

Sharding:
  - Phase 1+2 (QKV projection + attention): 2 heads per core. alibi is read
    fp8 (x256), 2 heads per core.
  - AllToAll (0.5 MiB/core, fp8) re-shards attention output heads->tokens.
  - Phase 3 (out-proj + residual + LayerNorm + MLP): 512 tokens per core.

Numerics (fp8e4m3 everywhere on the matmul paths, DoubleRow perf mode):
  - QKV proj: x fp8, w_qkv fp8 with sqrt(32) on q/k rows and 32 on v rows.
  - Scores psum = (a q)(a k) + 256*alibi = 256*(q.k/sqrt(Dh) + alibi);
    alibi (fp8, x256) is injected into the scores PSUM by identity-weight
    DoubleRow matmuls, then ScalarE computes exp(psum/256) -> fp8 directly.
  - attn@v runs fp8 DoubleRow over kt pairs; softmax denominator falls out
    of a 2.0-column appended to V (v is x32, so yn = 16*y after the
    reciprocal broadcast multiply).
  - Out-proj/MLP weights are x256 in fp8; residuals are pre-scaled x256 on
    host (LayerNorm is scale-invariant), and the final residual add is
    rescaled by 1/256 on ScalarE before the output DMA.

kernel(**inputs) takes FULL unsharded inputs, returns the FULL output.
"""

import sys

for _p in ("/opt/trn_rl_repo", "/root/.axon_site/_ro/trn_rl_repo"):
    if _p not in sys.path:
        sys.path.insert(0, _p)

import numpy as np
import ml_dtypes

import concourse.bass as bass
import concourse.tile as tile
from concourse import bacc, mybir
from concourse.bass_utils import run_bass_kernel_spmd
from concourse.masks import make_identity

BF16 = ml_dtypes.bfloat16
FP8 = ml_dtypes.float8_e4m3

B, T, D, H, Dh, FF = 2, 2048, 1024, 16, 64, 4096
NTOK = B * T            # 4096
NCORES = 8
CHUNK = NTOK // NCORES  # 512 tokens per core
HPC = H // NCORES       # 2 heads per core

F32 = mybir.dt.float32
F32R = mybir.dt.float32r
BF = mybir.dt.bfloat16
F8 = mybir.dt.float8e4
AF = mybir.ActivationFunctionType
DR = mybir.MatmulPerfMode.DoubleRow

# scaling scheme (see module docstring)
S_SCORE = 256.0                  # scores psum scale
ALPHA = float(np.sqrt(32.0))     # q and k row scale (ALPHA^2 = S_SCORE/8)
S_V = 32.0                       # v row scale
ONES_V = 2.0                     # ones column value -> yn = (S_V/ONES_V)*y
S_Y = S_V / ONES_V               # 16: yn scale
S_W = 256.0                      # out-proj/MLP weight scale

_COMPILED = None


def _build(sim1=False):
    nc = bacc.Bacc("TRN2", target_bir_lowering=False, debug=False,
                   num_devices=1 if sim1 else NCORES)

    # ---- kernel I/O (per core) ----
    xT_io = nc.dram_tensor("xT8", [D, NTOK], F8, kind="ExternalInput").ap()
    wqkvT_io = nc.dram_tensor("wqkvT8", [D, 384], F8, kind="ExternalInput").ap()
    alibiT_io = nc.dram_tensor("alibiT8", [HPC, T, T], F8,
                               kind="ExternalInput").ap()
    w_outT_io = nc.dram_tensor("w_outT8", [D, D], F8, kind="ExternalInput").ap()
    x_res_io = nc.dram_tensor("x_res_h", [CHUNK, D], F32, kind="ExternalInput").ap()
    x_res_b_io = nc.dram_tensor("x_res_b_h", [CHUNK, D], F32,
                                kind="ExternalInput").ap()
    # packed as [p, ff, kk, fin] = w_mlp_in_eff[ff*128+fin, kk*128+p]
    w_inP_io = nc.dram_tensor("w_inP", [128, 32, 8, 128], BF,
                              kind="ExternalInput").ap()
    b_inT_io = nc.dram_tensor("b_inT", [128, 32], F32, kind="ExternalInput").ap()
    # [f, 0, d] = fp8 hi(256*w_mo^T); [f, 1, d] = fp8 of 16x residual
    w_mlp_outT_io = nc.dram_tensor("w_moHL", [FF, 2, D], F8,
                                   kind="ExternalInput").ap()
    out_io = nc.dram_tensor("out", [CHUNK, D], F32, kind="ExternalOutput").ap()

    # ---- internal DRAM ----
    cc_send = nc.dram_tensor("cc_send", [D, CHUNK], F8)
    cc_recv = nc.dram_tensor("cc_recv", [D, CHUNK], F8)

    KT = T // 128   # 16 k-tiles per batch
    KP = KT // 2    # 8 kt pairs

    with tile.TileContext(nc) as tc:
        with tc.tile_pool(name="consts", bufs=1) as consts:
            identf = consts.tile([128, 128], F32, tag="identf")
            make_identity(nc, identf[:])
            identr = consts.tile([128, 128], F32R, tag="identr")
            nc.vector.tensor_copy(identr[:], identf[:])
            identb = consts.tile([128, 128], BF, tag="identb")
            nc.vector.tensor_copy(identb[:], identf[:])
            identA = consts.tile([128, 2, 128], F8, tag="identA")
            nc.vector.memset(identA[:], 0.0)
            nc.vector.tensor_copy(identA[:, 0, :], identf[:])
            identB = consts.tile([128, 2, 128], F8, tag="identB")
            nc.vector.memset(identB[:], 0.0)
            nc.vector.tensor_copy(identB[:, 1, :], identf[:])
            warm_f = consts.tile([128, 512], F32, tag="warm_f")
            nc.vector.memset(warm_f[:], 0.5)
            warm_rhs = consts.tile([128, 512], F32R, tag="warm_rhs")
            nc.vector.tensor_copy(warm_rhs[:], warm_f[:])

            # phase-3 weight tiles; DMAs are issued mid-attention so they
            # don't block the projection input DMAs on the sync queue
            wout8T = consts.tile([128, 8, 1024], F8, tag="wout8T")
            b_in = consts.tile([128, 32], F32, tag="b_in")

            with tc.tile_pool(name="qkv", bufs=1) as qkv:
                # per-(b,head) q/k in DoubleRow layout [32, 2, T]
                qh8 = [[qkv.tile([32, 2, T], F8, tag=f"qh{b}_{hl}",
                                 name=f"qh{b}_{hl}") for hl in range(2)]
                       for b in range(2)]
                kh8 = [[qkv.tile([32, 2, T], F8, tag=f"kh{b}_{hl}",
                                 name=f"kh{b}_{hl}") for hl in range(2)]
                       for b in range(2)]
                vs = []
                for b in range(2):
                    vb = qkv.tile([128, 16, 2, 128], F8, tag=f"v{b}",
                                  name=f"v{b}")
                    nc.vector.memset(vb[:, :, :, 64:65], ONES_V)
                    nc.vector.memset(vb[:, :, :, 65:128], 0.0)
                    vs.append(vb)
                # yn[hl][b*2+qc] covers tokens [b*T + qc*1024, ...): 16*y fp8
                yn = [[qkv.tile([64, 1024], F8, tag=f"yn{hl}_{i}",
                                name=f"yn{hl}_{i}") for i in range(4)]
                      for hl in range(2)]

                with nc.named_scope("attn"), \
                     tc.tile_pool(name="p1x", bufs=1) as p1x, \
                     tc.tile_pool(name="p1w", bufs=1) as p1w, \
                     tc.tile_pool(name="p1ps", bufs=1, space="PSUM") as p1ps, \
                     tc.tile_pool(name="p1t", bufs=2) as p1t, \
                     tc.tile_pool(name="p1pt", bufs=1, space="PSUM") as p1pt, \
                     tc.tile_pool(name="alb", bufs=24) as albp, \
                     tc.tile_pool(name="exps", bufs=3) as expp, \
                     tc.tile_pool(name="sps", bufs=2, space="PSUM") as spsp, \
                     tc.tile_pool(name="yups", bufs=1, space="PSUM") as yupp, \
                     tc.tile_pool(name="nrm", bufs=3) as nrmp:
                    wq = []
                    for kkp in range(4):
                        w = p1w.tile([128, 2, 384], F8, tag=f"wq{kkp}")
                        nc.sync.dma_start(
                            w[:],
                            wqkvT_io[kkp * 256:(kkp + 1) * 256, :].rearrange(
                                "(i p) m -> p i m", i=2))
                        wq.append(w)

                    def proj_pass(b):
                        with nc.named_scope(f"qkvproj{b}"):
                            xts = [p1x.tile([128, 2, 2048], F8,
                                            tag=f"xt{kkp}", name=f"xt{kkp}_{b}")
                                   for kkp in range(4)]
                            for cc4 in range(4):
                                for kkp in range(4):
                                    nc.sync.dma_start(
                                        xts[kkp][:, :, cc4 * 512:(cc4 + 1) * 512],
                                        xT_io[kkp * 256:(kkp + 1) * 256,
                                              b * 2048 + cc4 * 512:
                                              b * 2048 + (cc4 + 1) * 512]
                                        .rearrange("(i p) t -> p i t", i=2))
                            q8t = p1t.tile([128, 2048], F8, tag="q8t",
                                           name=f"q8t_{b}")
                            k8t = p1t.tile([128, 2048], F8, tag="k8t",
                                           name=f"k8t_{b}")
                            for t in range(4):
                                for m in range(3):   # q, k, v
                                    ps = p1ps.tile([128, 512], F32, tag="proj",
                                                   name=f"proj{b}_{t}_{m}")
                                    for kkp in range(4):
                                        nc.tensor.matmul(
                                            ps[:],
                                            wq[kkp][:, :, m * 128:(m + 1) * 128],
                                            xts[kkp][:, :, t * 512:(t + 1) * 512],
                                            start=(kkp == 0), stop=(kkp == 3),
                                            perf_mode=DR)
                                    if m == 0:
                                        nc.vector.tensor_copy(
                                            q8t[:, t * 512:(t + 1) * 512], ps[:])
                                    elif m == 1:
                                        nc.vector.tensor_copy(
                                            k8t[:, t * 512:(t + 1) * 512], ps[:])
                                    else:
                                        vt = p1t.tile([128, 512], BF, tag="vt",
                                                      name=f"vt{b}_{t}")
                                        nc.vector.tensor_copy(vt[:], ps[:])
                                        for j in range(4):
                                            ti = t * 4 + j
                                            pt = p1pt.tile([128, 128], BF,
                                                           tag="pt",
                                                           name=f"pt{b}_{ti}")
                                            nc.tensor.transpose(
                                                pt[:],
                                                vt[:, j * 128:(j + 1) * 128],
                                                identb[:])
                                            nc.vector.tensor_copy(
                                                vs[b][:, ti, :, 0:64],
                                                pt[:].rearrange(
                                                    "p (a b) -> p a b", a=2))
                                    # reshuffle q/k -> [32, 2, T] per head as
                                    # each t-chunk lands (partition moves via
                                    # SBUF->SBUF DMA)
                                    if m < 2:
                                        src = q8t if m == 0 else k8t
                                        dst = qh8 if m == 0 else kh8
                                        for hl in range(2):
                                            for i in range(2):
                                                p0 = hl * 64 + i * 32
                                                nc.sync.dma_start(
                                                    dst[b][hl][:, i,
                                                               t * 512:
                                                               (t + 1) * 512],
                                                    src[p0:p0 + 32,
                                                        t * 512:(t + 1) * 512])

                    for wi_ in range(14):
                        wps = p1pt.tile([128, 512], F32, tag="pt",
                                        name=f"warms{wi_}")
                        nc.tensor.matmul(wps[:], identr[:], warm_rhs[:],
                                         start=True, stop=True)
                    proj_pass(0)

                    al_cache = {}

                    def attn_pass(hl, qc, b):
                        if (hl, qc) not in al_cache:
                            al_cache[(hl, qc)] = [
                                albp.tile([128, 2, 1024], F8, tag="al",
                                          name=f"al{hl}_{qc}_{kp}")
                                for kp in range(KP)]
                        als = al_cache[(hl, qc)]
                        yu = yupp.tile([128, 1024], F32, tag="yu",
                                       name=f"yu{hl}_{qc}_{b}")
                        for kp in range(KP):
                            if b == 0:
                                nc.sync.dma_start(
                                    als[kp][:],
                                    alibiT_io[hl, kp * 256:(kp + 1) * 256,
                                              qc * 1024:(qc + 1) * 1024]
                                    .rearrange("(j p) q -> p j q", j=2))
                            ex = expp.tile([128, 2, 1024], F8, tag="ex",
                                           name=f"ex_{hl}_{qc}_{b}_{kp}")
                            for j in range(2):
                                kt = kp * 2 + j
                                sp = spsp.tile([128, 1024], F32, tag="sp",
                                               name=f"sp{hl}_{qc}_{b}_{kt}")
                                for h2 in range(2):
                                    sl = slice(h2 * 512, (h2 + 1) * 512)
                                    nc.tensor.matmul(
                                        sp[:, sl],
                                        kh8[b][hl][:, :,
                                                   kt * 128:(kt + 1) * 128],
                                        qh8[b][hl][:, :,
                                                   qc * 1024 + h2 * 512:
                                                   qc * 1024 + (h2 + 1) * 512],
                                        start=True, stop=False, perf_mode=DR)
                                    nc.tensor.matmul(
                                        sp[:, sl],
                                        identA[:] if j == 0 else identB[:],
                                        als[kp][:, :, sl],
                                        start=False, stop=True, perf_mode=DR)
                                nc.scalar.activation(ex[:, j, :], sp[:],
                                                     AF.Exp,
                                                     scale=1.0 / S_SCORE)
                            for h2 in range(2):
                                sl = slice(h2 * 512, (h2 + 1) * 512)
                                nc.tensor.matmul(
                                    yu[:, sl],
                                    vs[b][:, kp * 2:kp * 2 + 2, hl, :],
                                    ex[:, :, sl],
                                    start=(kp == 0), stop=(kp == KP - 1),
                                    perf_mode=DR)
                        rec = nrmp.tile([1, 1024], F32, tag="rec",
                                        name=f"rec{hl}_{qc}_{b}")
                        nc.vector.reciprocal(rec[:], yu[64:65, :])
                        bc = nrmp.tile([64, 1024], F32, tag="bc",
                                       name=f"bc{hl}_{qc}_{b}")
                        nc.gpsimd.partition_broadcast(bc[:], rec[:])
                        i = b * 2 + qc
                        nc.vector.tensor_mul(
                            yn[hl][i][:], yu[0:64, :], bc[:])
                        nc.sync.dma_start(
                            bass.AP(tensor=cc_send,
                                    offset=(2 * i * 128 + hl * 64) * 512,
                                    ap=[[512, 64], [128 * 512, 2], [1, 512]]),
                            yn[hl][i][:].rearrange("p (h c) -> p h c", h=2))

                    for hl in range(2):
                        for b in range(2):
                            for qc in range(2):
                                attn_pass(hl, qc, b)
                            if hl == 0 and b == 0:
                                # batch-1 projection overlaps the first two
                                # attention rounds (they only touch batch 0)
                                proj_pass(1)
                                nc.sync.dma_start(
                                    wout8T[:],
                                    w_outT_io.rearrange(
                                        "(kk p) n -> p kk n", p=128))
                                nc.sync.dma_start(b_in[:], b_inT_io[:])

                with nc.named_scope("a2a"):
                    if sim1:
                        nc.sync.dma_start(cc_recv[:], cc_send[:])
                    else:
                        nc.gpsimd.collective_compute(
                            "AllToAll", mybir.AluOpType.bypass,
                            replica_groups=[list(range(NCORES))],
                            ins=[cc_send[:]], outs=[cc_recv[:]])

            # ---------------- phase 3: out-proj + LN + MLP ----------------
            with nc.named_scope("mlp"), \
                 tc.tile_pool(name="p3acc", bufs=2, space="PSUM") as p3acc, \
                 tc.tile_pool(name="p3mo", bufs=4, space="PSUM") as p3mo, \
                 tc.tile_pool(name="p3pt", bufs=2, space="PSUM") as p3pt, \
                 tc.tile_pool(name="p3sb", bufs=1) as p3sb, \
                 tc.tile_pool(name="p3r", bufs=3) as p3r, \
                 tc.tile_pool(name="p3s", bufs=4) as p3s, \
                 tc.tile_pool(name="mlpw", bufs=8) as mlpw:
                for wi_ in range(40):
                    wps = p3pt.tile([128, 512], F32, tag="pt3",
                                    name=f"warm{wi_}")
                    nc.tensor.matmul(wps[:], identr[:], warm_rhs[:],
                                     start=True, stop=True)
                yrT = p3sb.tile([128, 8, 512], F8, tag="yrT")
                nc.scalar.dma_start(
                    yrT[:], bass.AP(tensor=cc_recv, offset=0,
                                    ap=[[512, 128], [128 * 512, 8], [1, 512]]))

                y_sb = p3sb.tile([128, 4, D], F32, tag="y_sb")
                y2_sb = p3sb.tile([128, 4, D], F32, tag="y2_sb")
                x_res_r = x_res_io.rearrange("(t p) d -> p t d", p=128)
                x_res_b_r = x_res_b_io.rearrange("(t p) d -> p t d", p=128)
                for tt in range(4):
                    xr = p3r.tile([128, D], F32, tag="xr")
                    nc.sync.dma_start(xr[:], x_res_r[:, tt, :])
                    xrb = p3r.tile([128, D], F32, tag="xrb")
                    nc.sync.dma_start(xrb[:], x_res_b_r[:, tt, :])
                    for dc in range(2):
                        ps = p3acc.tile([128, 512], F32, tag="acc")
                        for kkp in range(4):
                            nc.tensor.matmul(
                                ps[:],
                                yrT[:, 2 * kkp:2 * kkp + 2,
                                    tt * 128:(tt + 1) * 128],
                                wout8T[:, 2 * kkp:2 * kkp + 2,
                                       dc * 512:(dc + 1) * 512],
                                start=(kkp == 0), stop=(kkp == 3),
                                perf_mode=DR)
                        nc.vector.tensor_add(
                            y_sb[:, tt, dc * 512:(dc + 1) * 512], ps[:],
                            xr[:, dc * 512:(dc + 1) * 512])
                        nc.vector.tensor_add(
                            y2_sb[:, tt, dc * 512:(dc + 1) * 512], ps[:],
                            xrb[:, dc * 512:(dc + 1) * 512])

                # LayerNorm -> h_norm (bf16) -> transpose -> hT (D-major)
                hT = p3sb.tile([128, 8, 512], BF, tag="hT")
                for tt in range(4):
                    stats = p3s.tile([128, 2, 6], F32, tag="stats")
                    for g in range(2):
                        nc.vector.bn_stats(
                            stats[:, g, :],
                            y_sb[:, tt, g * 512:(g + 1) * 512])
                    mv = p3s.tile([128, 2], F32, tag="mv")
                    nc.vector.bn_aggr(mv[:], stats[:])
                    eps = p3s.tile([128, 1], F32, tag="eps")
                    nc.vector.memset(eps[:], 1e-5)
                    sd = p3s.tile([128, 1], F32, tag="sd")
                    nc.scalar.activation(sd[:], mv[:, 1:2], AF.Sqrt,
                                         bias=eps[:], scale=1.0)
                    rstd = p3s.tile([128, 1], F32, tag="rstd")
                    nc.vector.reciprocal(rstd[:], sd[:])
                    nb = p3s.tile([128, 1], F32, tag="nb")
                    nc.vector.tensor_mul(nb[:], mv[:, 0:1], rstd[:])
                    nb2 = p3s.tile([128, 1], F32, tag="nb2")
                    nc.scalar.mul(nb2[:], nb[:], -1.0)
                    hn = p3r.tile([128, D], BF, tag="hn")
                    nc.scalar.activation(hn[:], y_sb[:, tt, :], AF.Identity,
                                         bias=nb2[:], scale=rstd[:])
                    for dc in range(8):
                        pt = p3pt.tile([128, 128], BF, tag="pt3")
                        nc.tensor.transpose(
                            pt[:], hn[:, dc * 128:(dc + 1) * 128], identb[:])
                        nc.vector.tensor_copy(
                            hT[:, dc, tt * 128:(tt + 1) * 128], pt[:])

                # MLP in (bf16) + gelu -> hmT2 (Ff-major fp8 hi/lo pairs)
                hmT2 = p3sb.tile([128, 32, 2, 512], F8, tag="hmT2")
                for ff in range(32):
                    wi = mlpw.tile([128, 8, 128], BF, tag="wi")
                    nc.sync.dma_start(wi[:], w_inP_io[:, ff, :, :])
                    ps = p3acc.tile([128, 512], F32, tag="acc")
                    for kk in range(8):
                        nc.tensor.matmul(ps[:], wi[:, kk, :], hT[:, kk, :],
                                         start=(kk == 0), stop=(kk == 7))
                    nc.scalar.activation(hmT2[:, ff, 0, :], ps[:], AF.Gelu,
                                         bias=b_in[:, ff:ff + 1], scale=1.0)
                    nc.scalar.mul(hmT2[:, ff, 1, :], hmT2[:, ff, 0, :],
                                  0.0625)

                # MLP out + final residual (psum is 256x; rescale on ACT)
                out_r = out_io.rearrange("(t p) d -> p t d", p=128)
                for dc in range(2):
                    pss = [p3mo.tile([128, 512], F32, tag="mo",
                                     name=f"mo{dc}_{i}") for i in range(4)]
                    for ff in range(32):
                        wo2 = mlpw.tile([128, 2, 512], F8, tag="wo2")
                        nc.sync.dma_start(
                            wo2[:],
                            w_mlp_outT_io[ff * 128:(ff + 1) * 128, :,
                                          dc * 512:(dc + 1) * 512])
                        for tt in range(4):
                            nc.tensor.matmul(
                                pss[tt][:],
                                hmT2[:, ff, :, tt * 128:(tt + 1) * 128],
                                wo2[:],
                                start=(ff == 0), stop=(ff == 31),
                                perf_mode=DR)
                    for tt in range(4):
                        fs = p3s.tile([128, 512], F32, tag="fs")
                        nc.vector.tensor_add(
                            fs[:], pss[tt][:],
                            y2_sb[:, tt, dc * 512:(dc + 1) * 512])
                        fin = p3s.tile([128, 512], F32, tag="fin")
                        nc.scalar.mul(fin[:], fs[:], 1.0 / S_W)
                        nc.sync.dma_start(
                            out_r[:, tt, dc * 512:(dc + 1) * 512], fin[:])

    nc.compile()
    return nc


def _f8(a):
    return np.clip(a, -240.0, 240.0).astype(FP8)


def _host_prep(x, alibi, ln1_w, w_qkv, w_out, ln2_w, w_mlp_in, b_mlp_in,
               w_mlp_out, b_mlp_out):
    f32 = np.float32
    x = np.asarray(x, f32)
    x_flat = np.ascontiguousarray(x.reshape(NTOK, D))
    xT8 = _f8(np.ascontiguousarray(x_flat.T))
    w_qkv = np.asarray(w_qkv, f32)
    w_out = np.asarray(w_out, f32)
    w_mlp_in = np.asarray(w_mlp_in, f32)
    w_mlp_out = np.asarray(w_mlp_out, f32)
    b_mlp_in = np.asarray(b_mlp_in, f32)
    b_mlp_out = np.asarray(b_mlp_out, f32)
    ln2_w = np.asarray(ln2_w, f32)
    alibi = np.asarray(alibi, f32)

    w_outT8 = _f8((S_W / S_Y) * np.ascontiguousarray(w_out.T))
    w_in_eff = w_mlp_in * ln2_w[None, :]          # (FF, D)
    # packed [p, ff, kk, fin] = w_in_eff[ff*128+fin, kk*128+p], bf16
    w_inP = np.ascontiguousarray(
        w_in_eff.reshape(32, 128, 8, 128).transpose(3, 0, 2, 1)).astype(BF16)
    wmT = S_W * np.ascontiguousarray(w_mlp_out.T)          # (FF, D)
    wm_hi = _f8(wmT)
    wm_lo = _f8(16.0 * (wmT - wm_hi.astype(f32)))
    w_moHL = np.ascontiguousarray(np.stack([wm_hi, wm_lo], axis=1))
    b_inT = np.ascontiguousarray(b_mlp_in.reshape(32, 128).T)

    in_maps = []
    for c in range(NCORES):
        h0 = HPC * c
        qrows = ALPHA * w_qkv[h0 * Dh:(h0 + HPC) * Dh]
        krows = ALPHA * w_qkv[H * Dh + h0 * Dh:H * Dh + (h0 + HPC) * Dh]
        vrows = S_V * w_qkv[2 * H * Dh + h0 * Dh:2 * H * Dh + (h0 + HPC) * Dh]
        wqkvT8 = _f8(np.ascontiguousarray(
            np.concatenate([qrows, krows, vrows], 0).T))
        alibiT8 = _f8(S_SCORE * np.ascontiguousarray(
            np.transpose(alibi[0, h0:h0 + HPC], (0, 2, 1))))
        x_res = np.ascontiguousarray(x_flat[c * CHUNK:(c + 1) * CHUNK])
        x_res_h = S_W * x_res
        x_res_b_h = S_W * (x_res + b_mlp_out[None, :])
        in_maps.append({
            "xT8": xT8, "wqkvT8": wqkvT8, "alibiT8": alibiT8,
            "w_outT8": w_outT8, "x_res_h": x_res_h, "x_res_b_h": x_res_b_h,
            "w_inP": w_inP, "b_inT": b_inT, "w_moHL": w_moHL,
        })
    return in_maps


def _get_compiled():
    global _COMPILED
    if _COMPILED is None:
        _COMPILED = _build()
    return _COMPILED


def kernel(_trace=False, **inputs):
    nc = _get_compiled()
    in_maps = _host_prep(**inputs)
    res = None
    for attempt in range(3):
        try:
            res = run_bass_kernel_spmd(nc, in_maps,
                                       core_ids=list(range(NCORES)),
                                       trace=_trace)
            break
        except Exception:
            if attempt == 2:
                raise
    out = np.concatenate([res.results[c]["out"] for c in range(NCORES)], 0)
    out = out.reshape(B, T, D).astype(np.float32)
    if _trace:
        return out, res
    return out


# revision 39
# speedup vs baseline: 1.1711x; 1.0649x over previous
"""Trainium2 Bass kernel for nn_AttentionBlock (B=2, T=2048, D=1024, H=16,
Dh=64, Ff=4096), SPMD across 8 NeuronCores in one NEFF launch.

Sharding:
  - Phase 1+2 (QKV projection + attention): 2 heads per core. alibi is read
    fp8 (x256), 2 heads per core.
  - AllToAll (0.5 MiB/core, fp8) re-shards attention output heads->tokens.
  - Phase 3 (out-proj + residual + LayerNorm + MLP): 512 tokens per core.

Numerics (fp8e4m3 everywhere on the matmul paths, DoubleRow perf mode):
  - QKV proj: x fp8, w_qkv fp8 with sqrt(32) on q/k rows and 32 on v rows.
  - Scores psum = (a q)(a k) + 256*alibi = 256*(q.k/sqrt(Dh) + alibi);
    alibi (fp8, x256) is injected into the scores PSUM by identity-weight
    DoubleRow matmuls, then ScalarE computes exp(psum/256) -> fp8 directly.
  - attn@v runs fp8 DoubleRow over kt pairs; softmax denominator falls out
    of a 2.0-column appended to V (v is x32, so yn = 16*y after the
    reciprocal broadcast multiply).
  - Out-proj/MLP weights are x256 in fp8; residuals are pre-scaled x256 on
    host (LayerNorm is scale-invariant), and the final residual add is
    rescaled by 1/256 on ScalarE before the output DMA.

kernel(**inputs) takes FULL unsharded inputs, returns the FULL output.
"""

import sys

for _p in ("/opt/trn_rl_repo", "/root/.axon_site/_ro/trn_rl_repo"):
    if _p not in sys.path:
        sys.path.insert(0, _p)

import numpy as np
import ml_dtypes

import concourse.bass as bass
import concourse.tile as tile
from concourse import bacc, mybir
from concourse.bass_utils import run_bass_kernel_spmd
from concourse.masks import make_identity

BF16 = ml_dtypes.bfloat16
FP8 = ml_dtypes.float8_e4m3

B, T, D, H, Dh, FF = 2, 2048, 1024, 16, 64, 4096
NTOK = B * T            # 4096
NCORES = 8
CHUNK = NTOK // NCORES  # 512 tokens per core
HPC = H // NCORES       # 2 heads per core

F32 = mybir.dt.float32
F32R = mybir.dt.float32r
BF = mybir.dt.bfloat16
F8 = mybir.dt.float8e4
AF = mybir.ActivationFunctionType
DR = mybir.MatmulPerfMode.DoubleRow

# scaling scheme (see module docstring)
S_SCORE = 256.0                  # scores psum scale
ALPHA = float(np.sqrt(32.0))     # q and k row scale (ALPHA^2 = S_SCORE/8)
S_V = 32.0                       # v row scale
ONES_V = 2.0                     # ones column value -> yn = (S_V/ONES_V)*y
S_Y = S_V / ONES_V               # 16: yn scale
S_W = 256.0                      # out-proj/MLP weight scale

_COMPILED = None


def _build(sim1=False):
    nc = bacc.Bacc("TRN2", target_bir_lowering=False, debug=False,
                   num_devices=1 if sim1 else NCORES)

    # ---- kernel I/O (per core) ----
    xT_io = nc.dram_tensor("xT8", [D, NTOK], F8, kind="ExternalInput").ap()
    wqkvT_io = nc.dram_tensor("wqkvT8", [D, 384], F8, kind="ExternalInput").ap()
    alibiT_io = nc.dram_tensor("alibiT8", [HPC, T, T], F8,
                               kind="ExternalInput").ap()
    w_outT_io = nc.dram_tensor("w_outT8", [D, D], F8, kind="ExternalInput").ap()
    x_res_io = nc.dram_tensor("x_res_h", [CHUNK, D], F32, kind="ExternalInput").ap()
    b256_io = nc.dram_tensor("b256", [128, D], F32,
                             kind="ExternalInput").ap()
    # packed [p, ffp, kk, f, fin] = w_mlp_in_eff[(2*ffp+f)*128+fin, kk*128+p]
    w_inP_io = nc.dram_tensor("w_inP", [128, 16, 8, 2, 128], BF,
                              kind="ExternalInput").ap()
    b_inT_io = nc.dram_tensor("b_inT", [128, 32], F32, kind="ExternalInput").ap()
    # [f, 0, d] = fp8 hi(256*w_mo^T); [f, 1, d] = fp8 of 16x residual
    w_mlp_outT_io = nc.dram_tensor("w_moHL", [FF, 2, D], F8,
                                   kind="ExternalInput").ap()
    out_io = nc.dram_tensor("out", [CHUNK, D], F32, kind="ExternalOutput").ap()

    # ---- internal DRAM ----
    cc_send = nc.dram_tensor("cc_send", [D, CHUNK], F8)
    cc_recv = nc.dram_tensor("cc_recv", [D, CHUNK], F8)

    KT = T // 128   # 16 k-tiles per batch
    KP = KT // 2    # 8 kt pairs

    with tile.TileContext(nc) as tc:
        with tc.tile_pool(name="consts", bufs=1) as consts:
            identf = consts.tile([128, 128], F32, tag="identf")
            make_identity(nc, identf[:])
            identr = consts.tile([128, 128], F32R, tag="identr")
            nc.vector.tensor_copy(identr[:], identf[:])
            identb = consts.tile([128, 128], BF, tag="identb")
            nc.vector.tensor_copy(identb[:], identf[:])
            identA = consts.tile([128, 2, 128], F8, tag="identA")
            nc.vector.memset(identA[:], 0.0)
            nc.vector.tensor_copy(identA[:, 0, :], identf[:])
            identB = consts.tile([128, 2, 128], F8, tag="identB")
            nc.vector.memset(identB[:], 0.0)
            nc.vector.tensor_copy(identB[:, 1, :], identf[:])
            warm_f = consts.tile([128, 512], F32, tag="warm_f")
            nc.vector.memset(warm_f[:], 0.5)
            warm_rhs = consts.tile([128, 512], F32R, tag="warm_rhs")
            nc.vector.tensor_copy(warm_rhs[:], warm_f[:])

            # phase-3 weight tiles; DMAs are issued mid-attention so they
            # don't block the projection input DMAs on the sync queue
            wout8T = consts.tile([128, 8, 1024], F8, tag="wout8T")
            b_in = consts.tile([128, 32], F32, tag="b_in")


            with tc.tile_pool(name="qkv", bufs=1) as qkv:
                # per-(b,head) q/k in DoubleRow layout [32, 2, T]
                qh8 = [[qkv.tile([32, 2, T], F8, tag=f"qh{b}_{hl}",
                                 name=f"qh{b}_{hl}") for hl in range(2)]
                       for b in range(2)]
                kh8 = [[qkv.tile([32, 2, T], F8, tag=f"kh{b}_{hl}",
                                 name=f"kh{b}_{hl}") for hl in range(2)]
                       for b in range(2)]
                vs = []
                for b in range(2):
                    vb = qkv.tile([128, 16, 2, 128], F8, tag=f"v{b}",
                                  name=f"v{b}")
                    nc.vector.memset(vb[:, :, :, 64:65], ONES_V)
                    nc.vector.memset(vb[:, :, :, 65:128], 0.0)
                    vs.append(vb)
                # yn[hl][b*2+qc] covers tokens [b*T + qc*1024, ...): 16*y fp8
                yn = [[qkv.tile([64, 1024], F8, tag=f"yn{hl}_{i}",
                                name=f"yn{hl}_{i}") for i in range(4)]
                      for hl in range(2)]

                with nc.named_scope("attn"), \
                     tc.tile_pool(name="p1x", bufs=1) as p1x, \
                     tc.tile_pool(name="p1w", bufs=1) as p1w, \
                     tc.tile_pool(name="p1t", bufs=2) as p1t, \
                     tc.tile_pool(name="alb", bufs=12) as albp, \
                     tc.tile_pool(name="exps", bufs=4) as expp, \
                     tc.tile_pool(name="nrm", bufs=2) as nrmp:
                    wq = []
                    for kkp in range(4):
                        w = p1w.tile([128, 2, 384], F8, tag=f"wq{kkp}")
                        nc.sync.dma_start(
                            w[:],
                            wqkvT_io[kkp * 256:(kkp + 1) * 256, :].rearrange(
                                "(i p) m -> p i m", i=2))
                        wq.append(w)

                    def proj_pass(b, p1ps, p1pt):
                        with nc.named_scope(f"qkvproj{b}"):
                            xts = p1x.tile([128, 4, 2, 2048], F8,
                                           tag="xts", name=f"xts_{b}")
                            for cc4 in range(4):
                                for i in range(2):
                                    nc.sync.dma_start(
                                        xts[:, :, i,
                                            cc4 * 512:(cc4 + 1) * 512],
                                        xT_io[:, b * 2048 + cc4 * 512:
                                              b * 2048 + (cc4 + 1) * 512]
                                        .rearrange("(kkp i p) t -> p kkp i t",
                                                   kkp=4, i=2)[:, :, i, :])
                            q8t = p1t.tile([128, 2048], F8, tag="q8t",
                                           name=f"q8t_{b}")
                            k8t = p1t.tile([128, 2048], F8, tag="k8t",
                                           name=f"k8t_{b}")
                            for t in range(4):
                                for m in range(3):   # q, k, v
                                    ps = p1ps.tile([128, 512], F32, tag="proj",
                                                   name=f"proj{b}_{t}_{m}")
                                    for kkp in range(4):
                                        nc.tensor.matmul(
                                            ps[:],
                                            wq[kkp][:, :, m * 128:(m + 1) * 128],
                                            xts[:, kkp, :,
                                                t * 512:(t + 1) * 512],
                                            start=(kkp == 0), stop=(kkp == 3),
                                            perf_mode=DR)
                                    if m == 0:
                                        nc.vector.tensor_copy(
                                            q8t[:, t * 512:(t + 1) * 512], ps[:])
                                    elif m == 1:
                                        nc.vector.tensor_copy(
                                            k8t[:, t * 512:(t + 1) * 512], ps[:])
                                    else:
                                        vt = p1t.tile([128, 512], BF, tag="vt",
                                                      name=f"vt{b}_{t}")
                                        nc.vector.tensor_copy(vt[:], ps[:])
                                        for j in range(4):
                                            ti = t * 4 + j
                                            pt = p1pt.tile([128, 128], BF,
                                                           tag="pt",
                                                           name=f"pt{b}_{ti}")
                                            nc.tensor.transpose(
                                                pt[:],
                                                vt[:, j * 128:(j + 1) * 128],
                                                identb[:])
                                            nc.vector.tensor_copy(
                                                vs[b][:, ti, :, 0:64],
                                                pt[:].rearrange(
                                                    "p (a b) -> p a b", a=2))
                                    # reshuffle q -> [32, 2, T] per head as
                                    # each t-chunk lands (partition moves via
                                    # SBUF->SBUF DMA); k is batched below
                                    if m == 0:
                                        for hl in range(2):
                                            for i in range(2):
                                                p0 = hl * 64 + i * 32
                                                nc.sync.dma_start(
                                                    qh8[b][hl][:, i,
                                                               t * 512:
                                                               (t + 1) * 512],
                                                    q8t[p0:p0 + 32,
                                                        t * 512:(t + 1) * 512])
                            for hl in range(2):
                                for i in range(2):
                                    p0 = hl * 64 + i * 32
                                    nc.sync.dma_start(
                                        kh8[b][hl][:, i, :],
                                        k8t[p0:p0 + 32, :])

                    ctx0 = tc.tile_pool(name="p0ps", bufs=4, space="PSUM")
                    ctx0b = tc.tile_pool(name="p0pt", bufs=2, space="PSUM")
                    with ctx0 as p0ps, ctx0b as p0pt:
                        for wi_ in range(14):
                            wps = p0ps.tile([128, 512], F32, tag="proj",
                                            name=f"warms{wi_}")
                            nc.tensor.matmul(wps[:], identr[:], warm_rhs[:],
                                             start=True, stop=True)
                        proj_pass(0, p0ps, p0pt)

                    al_cache = {}
                    psum_pools = [None, None]

                    def attn_pass(hl, qc, b):
                        spsp, yupp = psum_pools
                        if (hl, qc) not in al_cache:
                            al_cache[(hl, qc)] = [
                                albp.tile([128, 4, 1024], F8, tag="al",
                                          name=f"al{hl}_{qc}_{kp2}")
                                for kp2 in range(KP // 2)]
                        als = al_cache[(hl, qc)]
                        yu = yupp.tile([128, 1024], F32, tag="yu",
                                       name=f"yu{hl}_{qc}_{b}")
                        for kp in range(KP):
                            alt = als[kp // 2]
                            ao = 2 * (kp % 2)
                            if b == 0 and kp % 2 == 0:
                                nc.sync.dma_start(
                                    alt[:],
                                    alibiT_io[hl, kp * 256:(kp + 2) * 256,
                                              qc * 1024:(qc + 1) * 1024]
                                    .rearrange("(j p) q -> p j q", j=4))
                            ex = expp.tile([128, 2, 1024], F8, tag="ex",
                                           name=f"ex_{hl}_{qc}_{b}_{kp}")
                            for j in range(2):
                                kt = kp * 2 + j
                                sp = spsp.tile([128, 1024], F32, tag="sp",
                                               name=f"sp{hl}_{qc}_{b}_{kt}")
                                for h2 in range(2):
                                    sl = slice(h2 * 512, (h2 + 1) * 512)
                                    nc.tensor.matmul(
                                        sp[:, sl],
                                        kh8[b][hl][:, :,
                                                   kt * 128:(kt + 1) * 128],
                                        qh8[b][hl][:, :,
                                                   qc * 1024 + h2 * 512:
                                                   qc * 1024 + (h2 + 1) * 512],
                                        start=True, stop=False, perf_mode=DR)
                                    nc.tensor.matmul(
                                        sp[:, sl],
                                        identA[:] if j == 0 else identB[:],
                                        alt[:, ao:ao + 2, sl],
                                        start=False, stop=True, perf_mode=DR)
                                nc.scalar.activation(ex[:, j, :], sp[:],
                                                     AF.Exp,
                                                     scale=1.0 / S_SCORE)
                            for h2 in range(2):
                                sl = slice(h2 * 512, (h2 + 1) * 512)
                                nc.tensor.matmul(
                                    yu[:, sl],
                                    vs[b][:, kp * 2:kp * 2 + 2, hl, :],
                                    ex[:, :, sl],
                                    start=(kp == 0), stop=(kp == KP - 1),
                                    perf_mode=DR)
                        rec = nrmp.tile([1, 1024], F32, tag="rec",
                                        name=f"rec{hl}_{qc}_{b}")
                        nc.vector.reciprocal(rec[:], yu[64:65, :])
                        bc = nrmp.tile([64, 1024], F32, tag="bc",
                                       name=f"bc{hl}_{qc}_{b}")
                        nc.gpsimd.partition_broadcast(bc[:], rec[:])
                        i = b * 2 + qc
                        nc.vector.tensor_mul(
                            yn[hl][i][:], yu[0:64, :], bc[:])
                        nc.sync.dma_start(
                            bass.AP(tensor=cc_send,
                                    offset=(2 * i * 128 + hl * 64) * 512,
                                    ap=[[512, 64], [128 * 512, 2], [1, 512]]),
                            yn[hl][i][:].rearrange("p (h c) -> p h c", h=2))

                    with tc.tile_pool(name="sps", bufs=2,
                                      space="PSUM") as spsp2, \
                         tc.tile_pool(name="yups", bufs=1,
                                      space="PSUM") as yupp2, \
                         tc.tile_pool(name="pj1ps", bufs=1,
                                      space="PSUM") as pj1ps, \
                         tc.tile_pool(name="pj1pt", bufs=1,
                                      space="PSUM") as pj1pt:
                        psum_pools[0] = spsp2
                        psum_pools[1] = yupp2
                        for hl in range(2):
                            for b in range(2):
                                for qc in range(2):
                                    attn_pass(hl, qc, b)
                                if hl == 0 and b == 0:
                                    # batch-1 projection overlaps the first
                                    # two attention rounds (batch 0 only)
                                    proj_pass(1, pj1ps, pj1pt)
                                    nc.sync.dma_start(
                                        wout8T[:],
                                        w_outT_io.rearrange(
                                            "(kk p) n -> p kk n", p=128))
                                    nc.sync.dma_start(b_in[:], b_inT_io[:])

                with nc.named_scope("a2a"):
                    if sim1:
                        nc.sync.dma_start(cc_recv[:], cc_send[:])
                    else:
                        nc.gpsimd.collective_compute(
                            "AllToAll", mybir.AluOpType.bypass,
                            replica_groups=[list(range(NCORES))],
                            ins=[cc_send[:]], outs=[cc_recv[:]])

            # ---------------- phase 3: out-proj + LN + MLP ----------------
            with nc.named_scope("mlp"), \
                 tc.tile_pool(name="p3acc", bufs=2, space="PSUM") as p3acc, \
                 tc.tile_pool(name="p3mo", bufs=4, space="PSUM") as p3mo, \
                 tc.tile_pool(name="p3pt", bufs=2, space="PSUM") as p3pt, \
                 tc.tile_pool(name="p3sb", bufs=1) as p3sb, \
                 tc.tile_pool(name="p3r", bufs=3) as p3r, \
                 tc.tile_pool(name="p3s", bufs=4) as p3s, \
                 tc.tile_pool(name="mlpw", bufs=3) as mlpw, \
                 tc.tile_pool(name="mow", bufs=8) as mow:
                for wi_ in range(8):
                    wps = p3pt.tile([128, 512], F32, tag="pt3",
                                    name=f"warm{wi_}")
                    nc.tensor.matmul(wps[:], identr[:], warm_rhs[:],
                                     start=True, stop=True)
                yrT = p3sb.tile([128, 8, 512], F8, tag="yrT")
                nc.scalar.dma_start(
                    yrT[:], bass.AP(tensor=cc_recv, offset=0,
                                    ap=[[512, 128], [128 * 512, 8], [1, 512]]))
                b256 = p3sb.tile([128, D], F32, tag="b256")
                nc.scalar.dma_start(b256[:], b256_io[:])

                y_sb = p3sb.tile([128, 4, D], F32, tag="y_sb")
                y2_sb = p3sb.tile([128, 4, D], F32, tag="y2_sb")
                x_res_r = x_res_io.rearrange("(t p) d -> p t d", p=128)
                hT = p3sb.tile([128, 8, 512], BF, tag="hT")
                for tt in range(4):
                    xr = p3r.tile([128, D], F32, tag="xr")
                    nc.sync.dma_start(xr[:], x_res_r[:, tt, :])
                    for dc in range(2):
                        ps = p3acc.tile([128, 512], F32, tag="acc")
                        for kkp in range(4):
                            nc.tensor.matmul(
                                ps[:],
                                yrT[:, 2 * kkp:2 * kkp + 2,
                                    tt * 128:(tt + 1) * 128],
                                wout8T[:, 2 * kkp:2 * kkp + 2,
                                       dc * 512:(dc + 1) * 512],
                                start=(kkp == 0), stop=(kkp == 3),
                                perf_mode=DR)
                        nc.vector.tensor_add(
                            y_sb[:, tt, dc * 512:(dc + 1) * 512], ps[:],
                            xr[:, dc * 512:(dc + 1) * 512])
                    stats = p3s.tile([128, 2, 6], F32, tag="stats")
                    for g in range(2):
                        nc.vector.bn_stats(
                            stats[:, g, :],
                            y_sb[:, tt, g * 512:(g + 1) * 512])
                    mv = p3s.tile([128, 2], F32, tag="mv")
                    nc.vector.bn_aggr(mv[:], stats[:])
                    eps = p3s.tile([128, 1], F32, tag="eps")
                    nc.vector.memset(eps[:], 1e-5)
                    sd = p3s.tile([128, 1], F32, tag="sd")
                    nc.scalar.activation(sd[:], mv[:, 1:2], AF.Sqrt,
                                         bias=eps[:], scale=1.0)
                    rstd = p3s.tile([128, 1], F32, tag="rstd")
                    nc.vector.reciprocal(rstd[:], sd[:])
                    nb = p3s.tile([128, 1], F32, tag="nb")
                    nc.vector.tensor_mul(nb[:], mv[:, 0:1], rstd[:])
                    nb2 = p3s.tile([128, 1], F32, tag="nb2")
                    nc.scalar.mul(nb2[:], nb[:], -1.0)
                    hn = p3r.tile([128, D], BF, tag="hn")
                    nc.scalar.activation(hn[:], y_sb[:, tt, :], AF.Identity,
                                         bias=nb2[:], scale=rstd[:])
                    for dc in range(8):
                        pt = p3pt.tile([128, 128], BF, tag="pt3")
                        nc.tensor.transpose(
                            pt[:], hn[:, dc * 128:(dc + 1) * 128], identb[:])
                        nc.vector.tensor_copy(
                            hT[:, dc, tt * 128:(tt + 1) * 128], pt[:])

                # MLP in (bf16) + gelu -> hmT2 (Ff-major fp8 hi/lo pairs);
                # MLP-out weight loads are interleaved into the same DMA
                # stream so neither starves the other
                def load_wo4(ffq):
                    wo4 = mow.tile([128, 4, 2, D], F8, tag="wo4",
                                   name=f"wo4_{ffq}")
                    nc.sync.dma_start(
                        wo4[:],
                        w_mlp_outT_io[ffq * 512:(ffq + 1) * 512, :, :]
                        .rearrange("(f p) i n -> p f i n", f=4))
                    return wo4

                def load_wi2(ffp):
                    wi2 = mlpw.tile([128, 8, 2, 128], BF, tag="wi",
                                    name=f"wi2_{ffp}")
                    nc.sync.dma_start(wi2[:], w_inP_io[:, ffp, :, :, :])
                    return wi2

                # head-start on the weight streams (after out-proj so its
                # PE start isn't barrier-gated on them)
                wi2s = {ffp: load_wi2(ffp) for ffp in range(3)}
                wo4s = [load_wo4(0), load_wo4(1)]
                hmT2 = p3sb.tile([128, 32, 2, 512], F8, tag="hmT2")
                for ff in range(32):
                    if ff % 2 == 0 and ff // 2 + 3 < 16:
                        wi2s[ff // 2 + 3] = load_wi2(ff // 2 + 3)
                    if ff % 4 == 2 and 2 + ff // 4 < 8:
                        wo4s.append(load_wo4(2 + ff // 4))
                    wi2 = wi2s[ff // 2]
                    ps = p3acc.tile([128, 512], F32, tag="acc")
                    for kk in range(8):
                        nc.tensor.matmul(ps[:], wi2[:, kk, ff % 2, :],
                                         hT[:, kk, :],
                                         start=(kk == 0), stop=(kk == 7))
                    nc.scalar.activation(hmT2[:, ff, 0, :], ps[:], AF.Gelu,
                                         bias=b_in[:, ff:ff + 1], scale=1.0)
                    nc.scalar.mul(hmT2[:, ff, 1, :], hmT2[:, ff, 0, :],
                                  0.0625)

                # y2 = y_sb + 256*b_mlp_out, off the critical out-proj window
                for tt in range(4):
                    nc.vector.tensor_add(
                        y2_sb[:, tt, :], y_sb[:, tt, :], b256[:])

                # MLP out + final residual (psum is 256x; rescale on ACT)
                out_r = out_io.rearrange("(t p) d -> p t d", p=128)
                for dc in range(2):
                    pss = [p3mo.tile([128, 512], F32, tag="mo",
                                     name=f"mo{dc}_{i}") for i in range(4)]
                    for ff in range(32):
                        for tt in range(4):
                            nc.tensor.matmul(
                                pss[tt][:],
                                hmT2[:, ff, :, tt * 128:(tt + 1) * 128],
                                wo4s[ff // 4][:, ff % 4, :,
                                              dc * 512:(dc + 1) * 512],
                                start=(ff == 0), stop=(ff == 31),
                                perf_mode=DR)
                    for tt in range(4):
                        fs = p3s.tile([128, 512], F32, tag="fs")
                        nc.vector.tensor_add(
                            fs[:], pss[tt][:],
                            y2_sb[:, tt, dc * 512:(dc + 1) * 512])
                        fin = p3s.tile([128, 512], F32, tag="fin")
                        nc.scalar.mul(fin[:], fs[:], 1.0 / S_W)
                        nc.sync.dma_start(
                            out_r[:, tt, dc * 512:(dc + 1) * 512], fin[:])

    nc.compile()
    return nc


def _f8(a):
    return np.clip(a, -240.0, 240.0).astype(FP8)


def _host_prep(x, alibi, ln1_w, w_qkv, w_out, ln2_w, w_mlp_in, b_mlp_in,
               w_mlp_out, b_mlp_out):
    f32 = np.float32
    x = np.asarray(x, f32)
    x_flat = np.ascontiguousarray(x.reshape(NTOK, D))
    xT8 = _f8(np.ascontiguousarray(x_flat.T))
    w_qkv = np.asarray(w_qkv, f32)
    w_out = np.asarray(w_out, f32)
    w_mlp_in = np.asarray(w_mlp_in, f32)
    w_mlp_out = np.asarray(w_mlp_out, f32)
    b_mlp_in = np.asarray(b_mlp_in, f32)
    b_mlp_out = np.asarray(b_mlp_out, f32)
    ln2_w = np.asarray(ln2_w, f32)
    alibi = np.asarray(alibi, f32)

    w_outT8 = _f8((S_W / S_Y) * np.ascontiguousarray(w_out.T))
    w_in_eff = w_mlp_in * ln2_w[None, :]          # (FF, D)
    # packed [p, ff, kk, fin] = w_in_eff[ff*128+fin, kk*128+p], bf16
    # [p, ffp, kk, f, fin] = w_in_eff[(2*ffp+f)*128+fin, kk*128+p]
    w_inP = np.ascontiguousarray(
        w_in_eff.reshape(16, 2, 128, 8, 128).transpose(4, 0, 3, 1, 2)
    ).astype(BF16)
    wmT = S_W * np.ascontiguousarray(w_mlp_out.T)          # (FF, D)
    wm_hi = _f8(wmT)
    wm_lo = _f8(16.0 * (wmT - wm_hi.astype(f32)))
    w_moHL = np.ascontiguousarray(np.stack([wm_hi, wm_lo], axis=1))
    b_inT = np.ascontiguousarray(b_mlp_in.reshape(32, 128).T)
    b256 = np.ascontiguousarray(
        np.broadcast_to(S_W * b_mlp_out[None, :], (128, D)))

    in_maps = []
    for c in range(NCORES):
        h0 = HPC * c
        qrows = ALPHA * w_qkv[h0 * Dh:(h0 + HPC) * Dh]
        krows = ALPHA * w_qkv[H * Dh + h0 * Dh:H * Dh + (h0 + HPC) * Dh]
        vrows = S_V * w_qkv[2 * H * Dh + h0 * Dh:2 * H * Dh + (h0 + HPC) * Dh]
        wqkvT8 = _f8(np.ascontiguousarray(
            np.concatenate([qrows, krows, vrows], 0).T))
        alibiT8 = _f8(S_SCORE * np.ascontiguousarray(
            np.transpose(alibi[0, h0:h0 + HPC], (0, 2, 1))))
        x_res = np.ascontiguousarray(x_flat[c * CHUNK:(c + 1) * CHUNK])
        x_res_h = S_W * x_res
        in_maps.append({
            "xT8": xT8, "wqkvT8": wqkvT8, "alibiT8": alibiT8,
            "w_outT8": w_outT8, "x_res_h": x_res_h, "b256": b256,
            "w_inP": w_inP, "b_inT": b_inT, "w_moHL": w_moHL,
        })
    return in_maps


def _get_compiled():
    global _COMPILED
    if _COMPILED is None:
        _COMPILED = _build()
    return _COMPILED


def kernel(_trace=False, **inputs):
    nc = _get_compiled()
    in_maps = _host_prep(**inputs)
    res = None
    for attempt in range(3):
        try:
            res = run_bass_kernel_spmd(nc, in_maps,
                                       core_ids=list(range(NCORES)),
                                       trace=_trace)
            break
        except Exception:
            if attempt == 2:
                raise
    out = np.concatenate([res.results[c]["out"] for c in range(NCORES)], 0)
    out = out.reshape(B, T, D).astype(np.float32)
    if _trace:
        return out, res
    return out
